# revision 1
# baseline (speedup 1.0000x reference)
"""Trainium2 Bass kernel for nn_EventTemplateBank (batched 1-D template-bank conv).

Math: score[b,t,e] = sum_{f,l} delayed[e,f,l] * x[b, t+40-l, f] / (L*F),
with delayed = delay-shifted templates (zero fill) and x zero-padded.

Device formulation (per core, data-parallel over batch):
  - Contract over a 128-position window on SBUF partitions.
  - Host pre-permutes x into overlapping-window scratch with one flat
    column axis across the core's 8 batches (683 columns per batch,
    zero-padded to 11*512):
        Xsc[k, f, c] = x[b, 48n + k - 39, f],  c = 683*b + n
    so every output t = 48n + D (D in [0,48)) has its full 80-tap window
    inside the k range of column c.
  - Toeplitz weights (host-built from the tiny templates):
        W[k, s, f, 16d+e] = delayed[e, f, (8s+d) + 79 - k] / 480
    One PSUM tile per D-set s accumulates 6 matmuls (one per feature f):
        out[s][m=(d,e), c-block] += W[:, s, f].T @ Xsc[:, f, c-block]
    Operands are float32r (single-pass PE, ~1 cycle/column at N=512).
  - Output written to DRAM in matmul-native layout; host re-permutes to (B,S,E).
"""

import numpy as np

import concourse.mybir as mybir
from concourse import bacc
from concourse.bass_utils import run_bass_kernel_spmd
from concourse.tile import TileContext

# Problem shapes (hardcoded per contract)
B, S, F = 64, 32768, 6
E, L = 16, 80
MAX_DELAY = 10

NCORES = 8
BPC = B // NCORES          # batches per core
Q = 48                     # output positions per rhs column
KWIN = 128                 # contraction window (partitions)
NS = 6                     # D-sets of 8 -> D in [0, 48)
PADF = 39                  # window of column n starts at 48n - 39
NCOLB = (S + Q - 1) // Q   # 683 columns per batch
BLKN = 512                 # columns per matmul block
NBLK = 11                  # ceil(8*683 / 512)
CPAD = NBLK * BLKN         # 5632 padded columns per core
CTOT = BPC * NCOLB         # 5464 real columns per core
LASTN = CTOT - (NBLK - 1) * BLKN   # 344 real columns in the last block

LAST_RESULT = None         # BassKernelResults of the most recent run (for profiling)


def _build_weights(templates: np.ndarray, onset_delays: np.ndarray) -> np.ndarray:
    """W[k, s, f, 16d+e] = delayed[e, f, (8s+d)+79-k] / (L*F), zero outside [0,L)."""
    d = np.round(np.clip(onset_delays, -MAX_DELAY, MAX_DELAY)).astype(np.int64)
    idx = np.arange(L)
    src = idx[None, None, :] - d[:, :, None]                 # (E,F,L)
    valid = (src >= 0) & (src < L)
    delayed = np.take_along_axis(templates, np.clip(src, 0, L - 1), axis=2)
    delayed = np.where(valid, delayed, 0.0).astype(np.float32) / float(L * F)

    D = (8 * np.arange(NS)[:, None] + np.arange(8)[None, :])      # (NS, 8)
    l_idx = D[:, :, None] + 79 - np.arange(KWIN)[None, None, :]   # (NS, 8, K)
    ok = (l_idx >= 0) & (l_idx < L)
    g = delayed[:, :, np.clip(l_idx, 0, L - 1)]                   # (E, F, NS, 8, K)
    g = np.where(ok[None, None], g, 0.0)
    # -> W[k, s, f, dd, e] (k-major so the device DMA is contiguous)
    W = g.transpose(4, 2, 1, 3, 0).reshape(KWIN, NS, F, 128)
    return np.ascontiguousarray(W, dtype=np.float32)


def _build_xsc(x: np.ndarray) -> np.ndarray:
    """Xsc[core, k, f, c] = x[8*core + c//683, 48*(c%683) + k - 39, f], zero OOB/pad."""
    need = Q * (NCOLB - 1) + KWIN
    xpad = np.zeros((B, PADF + need, F), dtype=np.float32)
    xpad[:, PADF:PADF + S, :] = x
    sb, st, sf = xpad.strides
    v = np.lib.stride_tricks.as_strided(
        xpad, shape=(B, KWIN, F, NCOLB), strides=(sb, st, sf, Q * st)
    )
    out = np.zeros((NCORES, KWIN, F, CPAD), dtype=np.float32)
    for b in range(B):
        core, i = divmod(b, BPC)
        out[core, :, :, i * NCOLB:(i + 1) * NCOLB] = v[b]
    return out


def _build_program():
    f32 = mybir.dt.float32
    f32r = mybir.dt.float32r
    nc = bacc.Bacc("TRN2", target_bir_lowering=False, debug=False)
    xsc = nc.dram_tensor("xsc", [KWIN, F, CPAD], f32, kind="ExternalInput")
    w = nc.dram_tensor("w", [KWIN, NS, F, 128], f32, kind="ExternalInput")
    osc = nc.dram_tensor("osc", [NBLK, NS, 128, BLKN], f32, kind="ExternalOutput")

    with TileContext(nc) as tc:
        with (
            tc.tile_pool(name="wp", bufs=1) as wp,
            tc.tile_pool(name="xp", bufs=20) as xp,
            tc.tile_pool(name="pp", bufs=8, space="PSUM") as pp,
            tc.tile_pool(name="op", bufs=6) as op,
        ):
            # Weights: per-set DMA + DVE cast-copy to float32r.
            wt_raw = wp.tile([KWIN, NS * F * 128], f32)
            wt = wp.tile([KWIN, NS * F * 128], f32r)
            wr = w.rearrange("k s f m -> k (s f m)")
            for s in range(NS):
                if s == 0:
                    # s=0 split per feature: the first matmul gates on a 64KB
                    # piece + 130ns cast instead of the whole 384KB set.
                    for f in range(F):
                        sl = slice(f * 128, (f + 1) * 128)
                        nc.sync.dma_start(out=wt_raw[:, sl], in_=wr[:, sl])
                        nc.vector.tensor_copy(out=wt[:, sl], in_=wt_raw[:, sl])
                else:
                    sl = slice(s * F * 128, (s + 1) * F * 128)
                    nc.sync.dma_start(out=wt_raw[:, sl], in_=wr[:, sl])
                    nc.vector.tensor_copy(out=wt[:, sl], in_=wt_raw[:, sl])
            for blk in range(NBLK):
                n = BLKN if blk < NBLK - 1 else LASTN
                # One SWDGE cast-DMA (f32 DRAM -> f32r SBUF) per feature plane:
                # matmuls gate on single 256KB planes, not the whole 1.5MB block.
                xtp = []
                for f in range(F):
                    xf = xp.tile([KWIN, n], f32r, tag="xtp")
                    nc.gpsimd.dma_start(
                        out=xf, in_=xsc[:, f, blk * BLKN:blk * BLKN + n]
                    )
                    xtp.append(xf)
                pss = [
                    pp.tile([128, n], f32, tag="ps", name=f"ps_{blk}_{s}")
                    for s in range(NS)
                ]

                def evac(s, n=n, blk=blk, pss=pss):
                    ot = op.tile([128, n], f32, tag="ot", name=f"ot_{blk}_{s}")
                    nc.vector.tensor_copy(out=ot, in_=pss[s])
                    nc.sync.dma_start(out=osc[blk, s, :, 0:n], in_=ot)

                if blk == 0:
                    # f-outer: each arriving x-plane feeds all 6 sets, so the
                    # PE starts as soon as the first 256KB plane lands.
                    for f in range(F):
                        for s in range(NS):
                            nc.tensor.matmul(
                                pss[s],
                                wt[:, (s * F + f) * 128:(s * F + f + 1) * 128],
                                xtp[f],
                                start=(f == 0),
                                stop=(f == F - 1),
                                skip_group_check=True,
                            )
                    for s in range(NS):
                        evac(s)
                else:
                    # s-outer: sets complete one after another, so PSUM
                    # evacuation + output DMA stagger across the block.
                    for s in range(NS):
                        for f in range(F):
                            nc.tensor.matmul(
                                pss[s],
                                wt[:, (s * F + f) * 128:(s * F + f + 1) * 128],
                                xtp[f],
                                start=(f == 0),
                                stop=(f == F - 1),
                            )
                        evac(s)
    nc.compile()   # bacc passes: split multi-waits (HW allows 1 wait/inst), DCE, reg alloc
    return nc


def kernel(x: np.ndarray, templates: np.ndarray, onset_delays: np.ndarray) -> np.ndarray:
    global LAST_RESULT
    x = np.ascontiguousarray(x, dtype=np.float32)
    templates = np.asarray(templates, dtype=np.float32)
    onset_delays = np.asarray(onset_delays, dtype=np.float32)

    W = _build_weights(templates, onset_delays)
    Xsc = _build_xsc(x)                                   # (NCORES, K, F, CPAD)

    nc = _build_program()
    in_maps = [{"xsc": Xsc[c], "w": W} for c in range(NCORES)]
    res = run_bass_kernel_spmd(nc, in_maps, core_ids=list(range(NCORES)))
    LAST_RESULT = res

    osc = np.stack([r["osc"] for r in res.results], axis=0)   # (NCORES,NBLK,NS,128,BLKN)
    o = osc.reshape(NCORES, NBLK, NS, 8, E, BLKN)             # core, blk, s, d, e, n
    o = o.transpose(0, 1, 5, 2, 3, 4)                          # core, blk, n, s, d, e
    o = np.ascontiguousarray(o).reshape(NCORES, CPAD, NS * 8 * E)
    o = o[:, :BPC * NCOLB, :].reshape(NCORES, BPC, NCOLB, NS, 8, E)
    o = o.reshape(B, NCOLB * Q, E)[:, :S, :]
    o = np.ascontiguousarray(o)
    o[:, S - 1, :] = 0.0                                   # reference zero-pads last column
    return o



# revision 2
# speedup vs baseline: 1.3747x; 1.3747x over previous
"""Trainium2 Bass kernel for nn_EventTemplateBank (batched 1-D template-bank conv).

Math: score[b,t,e] = sum_{f,l} delayed[e,f,l] * x[b, t+40-l, f] / (L*F),
with delayed = delay-shifted templates (zero fill) and x zero-padded.

Device formulation (per core, data-parallel over batch):
  - Interleaved-slot contraction: the 128-position x-window of an output
    column is stored feature-interleaved as 768 flat slots
    (slot = 6*window_pos + feature), chunked into 6 SBUF tiles of 128
    partitions. This is a contiguous 768-element run of flat x[b,(s,f)]
    memory per column, so the host scratch is one strided view:
        Xsc[tile, p, c] = xflat[b, 288*n - 234 + 128*tile + p],
        c = 683*b + n  (683 columns of 48 outputs per batch).
  - Each PSUM set s (outputs t = 48n + 8s + dd, dd in [0,8)) needs taps
    spanning 87 window positions = 522 consecutive slots, which always
    fits in 5 consecutive tiles: sets 0-2 use tiles 0-4, sets 3-5 use
    tiles 1-5. 30 matmuls per 512-column block instead of the 36 a
    per-feature window layout needs (75% vs 62.5% PE efficiency).
  - Everything bf16 (x scratch, Toeplitz weights, output): halves HBM
    traffic vs fp32 so the 16 SDMA engines stay off the critical path;
    PSUM accumulation is fp32.
  - Output written to DRAM in matmul-native layout; host re-permutes.
"""

import numpy as np
import ml_dtypes

import concourse.mybir as mybir
from concourse import bacc
from concourse.bass_utils import run_bass_kernel_spmd
from concourse.tile import TileContext

# Problem shapes (hardcoded per contract)
B, S, F = 64, 32768, 6
E, L = 16, 80
MAX_DELAY = 10

NCORES = 8
BPC = B // NCORES          # batches per core
Q = 48                     # output positions per rhs column
KWIN = 128                 # window positions per column
NTILE = 6                  # 768 slots = 6 tiles of 128 partitions
TPS = 5                    # tiles per set (522 slots span 5 tiles)
NS = 6                     # d-sets of 8 -> D in [0, 48)
PADF = 39                  # window of column n starts at 48n - 39
NCOLB = (S + Q - 1) // Q   # 683 columns per batch
BLKN = 512                 # columns per matmul block
NBLK = 11                  # ceil(8*683 / 512)
CPAD = NBLK * BLKN         # 5632 padded columns per core
CTOT = BPC * NCOLB         # 5464 real columns per core
LASTN = CTOT - (NBLK - 1) * BLKN   # 344 real columns in the last block

BF16 = ml_dtypes.bfloat16
LAST_RESULT = None         # BassKernelResults of the most recent run (for profiling)


def _tile_of(s: int, tl: int) -> int:
    return tl + (1 if s >= 3 else 0)


def _build_weights(templates: np.ndarray, onset_delays: np.ndarray) -> np.ndarray:
    """W[p, s, tl, 16dd+e] = delayed[e, f, (8s+dd)+79-k] / (L*F), zero outside [0,L),
    where (k, f) = divmod(128*tile_of(s,tl) + p, 6)."""
    d = np.round(np.clip(onset_delays, -MAX_DELAY, MAX_DELAY)).astype(np.int64)
    idx = np.arange(L)
    src = idx[None, None, :] - d[:, :, None]                 # (E,F,L)
    valid = (src >= 0) & (src < L)
    delayed = np.take_along_axis(templates, np.clip(src, 0, L - 1), axis=2)
    delayed = np.where(valid, delayed, 0.0).astype(np.float32) / float(L * F)

    W = np.zeros((KWIN, NS, TPS, 128), dtype=np.float32)
    dd = np.arange(8)
    for s in range(NS):
        for tl in range(TPS):
            slot = 128 * _tile_of(s, tl) + np.arange(128)
            k = slot // F
            f = slot % F
            l = (8 * s + dd)[None, :] + 79 - k[:, None]      # (128, 8)
            ok = (l >= 0) & (l < L)
            g = delayed[:, f[:, None], np.clip(l, 0, L - 1)]  # (E, 128, 8)
            g = np.where(ok[None], g, 0.0)
            W[:, s, tl, :] = g.transpose(1, 2, 0).reshape(128, 128)
    return np.ascontiguousarray(W.astype(BF16))


def _build_xsc(x: np.ndarray) -> np.ndarray:
    """Xsc[core, t, p, c] = xflat[b, 288*(c%683) + 128*t + p] (window starts at -234)."""
    need = Q * (NCOLB - 1) + KWIN
    xpad = np.zeros((B, PADF + need, F), dtype=BF16)
    xpad[:, PADF:PADF + S, :] = x.astype(BF16)
    xflat = np.ascontiguousarray(xpad.reshape(B, -1))
    ez = xflat.strides[1]
    v = np.lib.stride_tricks.as_strided(
        xflat, shape=(B, NTILE, KWIN, NCOLB),
        strides=(xflat.strides[0], 128 * ez, ez, Q * F * ez),
    )
    out = np.zeros((NCORES, NTILE, KWIN, CPAD), dtype=BF16)
    for b in range(B):
        core, i = divmod(b, BPC)
        out[core, :, :, i * NCOLB:(i + 1) * NCOLB] = v[b]
    return out


def _build_program():
    f32 = mybir.dt.float32
    bf16 = mybir.dt.bfloat16
    nc = bacc.Bacc("TRN2", target_bir_lowering=False, debug=False)
    xsc = nc.dram_tensor("xsc", [NTILE, KWIN, CPAD], bf16, kind="ExternalInput")
    w = nc.dram_tensor("w", [KWIN, NS, TPS, 128], bf16, kind="ExternalInput")
    osc = nc.dram_tensor("osc", [NBLK, NS, 128, BLKN], bf16, kind="ExternalOutput")

    with TileContext(nc) as tc:
        with (
            tc.tile_pool(name="wp", bufs=1) as wp,
            tc.tile_pool(name="xp", bufs=24) as xp,
            tc.tile_pool(name="pp", bufs=8, space="PSUM") as pp,
            tc.tile_pool(name="op", bufs=6) as op,
        ):
            # Weights land directly as bf16 — no cast pass needed.
            wt = wp.tile([KWIN, NS * TPS * 128], bf16)
            wr = w.rearrange("p s t m -> p (s t m)")
            for s in range(NS):
                if s == 0:
                    # s=0 split per tile: the first matmul gates on 32KB.
                    for tl in range(TPS):
                        sl = slice(tl * 128, (tl + 1) * 128)
                        nc.sync.dma_start(out=wt[:, sl], in_=wr[:, sl])
                else:
                    sl = slice(s * TPS * 128, (s + 1) * TPS * 128)
                    nc.sync.dma_start(out=wt[:, sl], in_=wr[:, sl])

            for blk in range(NBLK):
                n = BLKN if blk < NBLK - 1 else LASTN
                xtp = []
                for t in range(NTILE):
                    xf = xp.tile([KWIN, n], bf16, tag="xtp")
                    nc.gpsimd.dma_start(
                        out=xf, in_=xsc[t, :, blk * BLKN:blk * BLKN + n]
                    )
                    xtp.append(xf)
                pss = [
                    pp.tile([128, n], f32, tag="ps", name=f"ps_{blk}_{s}")
                    for s in range(NS)
                ]

                def evac(s, n=n, blk=blk, pss=pss):
                    ot = op.tile([128, n], bf16, tag="ot", name=f"ot_{blk}_{s}")
                    nc.vector.tensor_copy(out=ot, in_=pss[s])
                    nc.sync.dma_start(out=osc[blk, s, :, 0:n], in_=ot)

                if blk == 0:
                    # tile-major: each arriving x tile feeds every set using
                    # it, so the PE starts as soon as tile 0 lands.
                    for t in range(NTILE):
                        for s in range(NS):
                            tl = t - (1 if s >= 3 else 0)
                            if 0 <= tl < TPS:
                                nc.tensor.matmul(
                                    pss[s],
                                    wt[:, (s * TPS + tl) * 128:(s * TPS + tl + 1) * 128],
                                    xtp[t],
                                    start=(tl == 0),
                                    stop=(tl == TPS - 1),
                                    skip_group_check=True,
                                )
                    for s in range(NS):
                        evac(s)
                else:
                    # set-major: sets complete one after another, so PSUM
                    # evacuation + output DMA stagger across the block.
                    for s in range(NS):
                        g = 1 if s >= 3 else 0
                        for tl in range(TPS):
                            nc.tensor.matmul(
                                pss[s],
                                wt[:, (s * TPS + tl) * 128:(s * TPS + tl + 1) * 128],
                                xtp[tl + g],
                                start=(tl == 0),
                                stop=(tl == TPS - 1),
                            )
                        evac(s)
    nc.compile()
    return nc


def kernel(x: np.ndarray, templates: np.ndarray, onset_delays: np.ndarray) -> np.ndarray:
    global LAST_RESULT
    x = np.ascontiguousarray(x, dtype=np.float32)
    templates = np.asarray(templates, dtype=np.float32)
    onset_delays = np.asarray(onset_delays, dtype=np.float32)

    W = _build_weights(templates, onset_delays)
    Xsc = _build_xsc(x)                                   # (NCORES, NTILE, K, CPAD)

    nc = _build_program()
    in_maps = [{"xsc": Xsc[c], "w": W} for c in range(NCORES)]
    res = run_bass_kernel_spmd(nc, in_maps, core_ids=list(range(NCORES)))
    LAST_RESULT = res

    osc = np.stack([np.asarray(r["osc"]) for r in res.results], axis=0)
    o = osc.astype(np.float32)                                 # (NCORES,NBLK,NS,128,BLKN)
    o = o.reshape(NCORES, NBLK, NS, 8, E, BLKN)                # core, blk, s, dd, e, n
    o = o.transpose(0, 1, 5, 2, 3, 4)                          # core, blk, n, s, dd, e
    o = np.ascontiguousarray(o).reshape(NCORES, CPAD, NS * 8 * E)
    o = o[:, :BPC * NCOLB, :].reshape(NCORES, BPC, NCOLB, NS, 8, E)
    o = o.reshape(B, NCOLB * Q, E)[:, :S, :]
    o = np.ascontiguousarray(o)
    o[:, S - 1, :] = 0.0                                   # reference zero-pads last column
    return o
